# revision 9
# baseline (speedup 1.0000x reference)
"""AttLIF Trainium2 kernel v2: Linear(1024->2048) + temporal-attention gate +
IF-neuron scan.  B=256, T=64, DIN=1024, DH=2048, 8 cores, batch-parallel
(32 batches/core).

Structure (per core), all in the TRANSPOSED gemm layout (weights stationary):
  psum[h=128, bt=512] = sum_k WE[k, h-blk] @ dE^T[k, bt-grp]   (bf16 hi/lo x3)
  XT[p=h, col = hb*512 + b*64 + t] <- psum copies (ACT)        x in scan layout
  mx: running max over the 16 hb slices (+bias), PE-transpose, free-reduce
  avg: wavg-pair gemm (exact row-mean incl bias), PE-transpose
  score = sigmoid(W2 @ (relu(W1@avg)+relu(W1@mx)))  tiny fp32 PE matmuls,
          broadcast to 128 partitions with a K=1 ones matmul
  XT = (XT + bias) * score                                     DVE, per hb
  scan: u = x + v; v = u*(u<0.6)       DVE, T seq, [128,(16,8)] strided APs
  spikes = (XT >= 0.6) -> bf16, PE-transposed back to [bt, h] and stored as
  128-partition x 4KB-chunk DMAs.  Host casts bf16 spikes to f32.

vs v1: no SBUF->SBUF scatter (was 16MB of 2-partition DMAs), no 512B-chunk
output stores, NK=24 (bias row dropped; bias folded into DVE ops since the
partition dim is now h).
"""
import sys
from contextlib import ExitStack

import numpy as np

sys.path.insert(0, "/opt/trn_rl_repo")

VTH = 0.6
B, T, DIN, DH = 256, 64, 1024, 2048
NCORES = 8
BS = B // NCORES       # 32 batches per core
BG = 8                 # batches per group
NG = BS // BG          # 4 groups
NBT = BG * T           # 512 bt rows per group
NK = 24                # k-tiles: [dhi(8)|dhi(8)|dlo(8)] x [Whi|Wlo|Whi]
NKS = 16               # stored k-tiles (hi/lo dedup, both operands)
NHB = DH // 128        # 16 h-blocks
MODE = "v2"            # kept for test.py compat
NCC = 2                # bt chunks of 128 per batch pair


def _dcol(k):          # stored data tile for gemm k-step
    return k if k < 8 else k - 8


def _wcol(k):          # stored weight tile for gemm k-step
    return k if k < 16 else k - 16


def _build(nc, tile, mybir):
    from concourse.masks import make_identity

    f32 = mybir.dt.float32
    bf16 = mybir.dt.bfloat16
    aop = mybir.AluOpType
    act = mybir.ActivationFunctionType

    dT = nc.dram_tensor("dT", [NKS, 128, BS * T], bf16, kind="ExternalInput").ap()
    wT = nc.dram_tensor("wT", [NHB, 128, NKS * 128], bf16, kind="ExternalInput").ap()
    wavgT = nc.dram_tensor("wavgT", [128, NKS], bf16, kind="ExternalInput").ap()
    biasT = nc.dram_tensor("biasT", [128, NHB], f32, kind="ExternalInput").ap()
    mbT = nc.dram_tensor("mbT", [128, 1], f32, kind="ExternalInput").ap()
    w1t = nc.dram_tensor("w1t", [T, 4], f32, kind="ExternalInput").ap()
    w2t = nc.dram_tensor("w2t", [4, T], f32, kind="ExternalInput").ap()
    out = nc.dram_tensor("out", [BS, T, DH], bf16, kind="ExternalOutput").ap()

    with tile.TileContext(nc) as tc, ExitStack() as ctx:
        cpool = ctx.enter_context(tc.tile_pool(name="cpool", bufs=1))
        dpool = ctx.enter_context(tc.tile_pool(name="dpool", bufs=2 * NKS))
        xtpool = ctx.enter_context(tc.tile_pool(name="xtpool", bufs=2))
        xspool = ctx.enter_context(tc.tile_pool(name="xspool", bufs=2))
        ospool = ctx.enter_context(tc.tile_pool(name="ospool", bufs=1))
        m1pool = ctx.enter_context(tc.tile_pool(name="m1pool", bufs=1))
        stpool = ctx.enter_context(tc.tile_pool(name="stpool", bufs=1))
        scpool = ctx.enter_context(tc.tile_pool(name="scpool", bufs=1))
        vpool = ctx.enter_context(tc.tile_pool(name="vpool", bufs=1))
        pgemm = ctx.enter_context(tc.tile_pool(name="pgemm", bufs=2, space="PSUM"))
        ptr = ctx.enter_context(tc.tile_pool(name="ptr", bufs=2, space="PSUM"))
        pssc = ctx.enter_context(tc.tile_pool(name="pssc", bufs=1, space="PSUM"))
        pmisc = ctx.enter_context(tc.tile_pool(name="pmisc", bufs=1, space="PSUM"))

        # ---- constants / weights resident in SBUF ----
        ident_bf = cpool.tile([128, 128], bf16, name="ident_bf")
        make_identity(nc, ident_bf[:])
        ident_f = cpool.tile([128, 128], f32, name="ident_f")
        make_identity(nc, ident_f[:])
        ones_f = cpool.tile([1, 128], f32, name="ones_f")
        nc.vector.memset(ones_f[:], 1.0)

        w1t_sb = cpool.tile([128, 4], f32, name="w1t_sb")
        nc.sync.dma_start(w1t_sb[0:T, :], w1t[:])
        nc.sync.dma_start(w1t_sb[T:128, :], w1t[:])
        w2t_sb = cpool.tile([4, T], f32, name="w2t_sb")
        nc.sync.dma_start(w2t_sb[:], w2t[:])
        bias_sb = cpool.tile([128, NHB], f32, name="bias_sb")
        nc.sync.dma_start(bias_sb[:], biasT[:])
        mb_sb = cpool.tile([128, 1], f32, name="mb_sb")
        nc.sync.dma_start(mb_sb[:], mbT[:])
        # wavg pairs: cols 2*j   = bf16-hi of wavg k-range j (j = kt % 8)
        #             cols 2*j+1 = bf16-lo
        wavg_sb = cpool.tile([128, NKS], bf16, name="wavg_sb")
        nc.sync.dma_start(wavg_sb[:], wavgT[:])

        wcs = []
        for hb in range(NHB):
            wc = cpool.tile([128, NKS * 128], bf16, name=f"wc{hb}")
            (nc.sync if hb % 2 == 0 else nc.gpsimd).dma_start(wc[:], wT[hb])
            wcs.append(wc)

        # ---- per-group state (python handles; tiles cycle via pool tags) ----
        XTs = [None] * NG
        XSs = [None] * NG
        dts = [None] * NG

        def emit_loads(g):
            tiles = []
            for kt in range(NKS):
                dt = dpool.tile([128, NBT], bf16, name="dt", tag="dt")
                nc.gpsimd.dma_start(dt[:], dT[kt, :, g * NBT:(g + 1) * NBT])
                tiles.append(dt)
            dts[g] = tiles

        def emit_gemm(g):
            XT = xtpool.tile([128, NHB * NBT], f32, name="XT", tag="XT")
            XTs[g] = XT
            M1 = m1pool.tile([128, NBT], f32, name="M1", tag="M1")
            nc.vector.memset(M1[:], -1e30)
            dtg = dts[g]
            for hb in range(NHB):
                ps = pgemm.tile([128, NBT], f32, name="ps", tag="ps")
                wc = wcs[hb]
                for k in range(NK):
                    kw = _wcol(k)
                    nc.tensor.matmul(ps[:], wc[:, kw * 128:(kw + 1) * 128],
                                     dtg[_dcol(k)][:],
                                     start=(k == 0), stop=(k == NK - 1))
                nc.scalar.activation(XT[:, hb * NBT:(hb + 1) * NBT], ps[:],
                                     act.Copy)
                # running max over hb of (x + bias_hb)
                nc.vector.scalar_tensor_tensor(
                    M1[:], XT[:, hb * NBT:(hb + 1) * NBT],
                    bias_sb[:, hb:hb + 1], M1[:], op0=aop.add, op1=aop.max)
            return M1

        def emit_stats_score(g, M1):
            XT = XTs[g]
            dtg = dts[g]
            # avg = d @ wavg (hi+lo pair), exact incl bias via +mean(bias)
            pavg = pmisc.tile([2, NBT], f32, name="pavg", tag="pavg")
            for kt in range(NKS):
                j = kt % 8
                nc.tensor.matmul(pavg[:], wavg_sb[:, 2 * j:2 * j + 2],
                                 dtg[kt][:],
                                 start=(kt == 0), stop=(kt == NKS - 1))
            avg_sb = stpool.tile([2, NBT], f32, name="avg_sb", tag="avg_sb")
            nc.scalar.activation(avg_sb[:], pavg[:], act.Copy)

            # stats[p = b_l*64 + t, c*2 + {0: avg, 1: mx}] for b-pair c
            stats = stpool.tile([128, 2 * (BG // 2)], f32, name="stats",
                                tag="stats")
            for c in range(BG // 2):
                pat = pmisc.tile([128, 2], f32, name="pat", tag="pm2")
                nc.tensor.transpose(pat[:], avg_sb[:, c * 128:(c + 1) * 128],
                                    ident_f[0:2, 0:2])
                pats = scpool.tile([128, 2], f32, name="pats", tag="pats")
                nc.scalar.activation(pats[:], pat[:], act.Copy)
                # avg = hi_part + mean(bias) + lo_part
                nc.vector.scalar_tensor_tensor(
                    stats[:, 2 * c:2 * c + 1], pats[:, 0:1], mb_sb[:, 0:1],
                    pats[:, 1:2], op0=aop.add, op1=aop.add)
                pmt = pmisc.tile([128, 128], f32, name="pmt", tag="pm1")
                nc.tensor.transpose(pmt[:], M1[:, c * 128:(c + 1) * 128],
                                    ident_f[:])
                nc.vector.tensor_reduce(stats[:, 2 * c + 1:2 * c + 2], pmt[:],
                                        mybir.AxisListType.X, aop.max)

            # mlp: h1 = relu(W1 @ v) for v in {avg, mx}; Ht[r, b] summed
            h1a = pmisc.tile([4, 2 * (BG // 2)], f32, name="h1a", tag="pm1")
            nc.tensor.matmul(h1a[:], w1t_sb[0:T, :], stats[0:T, :],
                             start=True, stop=True)
            h1b = pmisc.tile([4, 2 * (BG // 2)], f32, name="h1b", tag="pm2")
            nc.tensor.matmul(h1b[:], w1t_sb[T:128, :], stats[T:128, :],
                             start=True, stop=True)
            h1r = scpool.tile([4, 4 * (BG // 2)], f32, name="h1r", tag="h1r")
            nc.scalar.activation(h1r[:, 0:8], h1a[:], act.Relu)
            nc.scalar.activation(h1r[:, 8:16], h1b[:], act.Relu)
            # h1r col = half*8 + c*2 + pair;  pair 0 = avg-part, 1 = mx-part
            # Ht[r, b]: b = 2c + half (even batches from h1a, odd from h1b)
            Ht = scpool.tile([4, BG], f32, name="Ht", tag="Ht")
            htv = Ht[:].rearrange("r (c two) -> r two c", two=2)
            h1v = h1r[:].rearrange("r (half c pair) -> r half c pair",
                                   half=2, pair=2)
            nc.vector.tensor_tensor(htv[:, 0, :], h1v[:, 0, :, 0],
                                    h1v[:, 0, :, 1], aop.add)
            nc.vector.tensor_tensor(htv[:, 1, :], h1v[:, 1, :, 0],
                                    h1v[:, 1, :, 1], aop.add)
            # score[b, t] then flatten to one partition and broadcast to 128
            spT = pmisc.tile([BG, T], f32, name="spT", tag="pm1")
            nc.tensor.matmul(spT[:], Ht[:], w2t_sb[:], start=True, stop=True)
            scb = scpool.tile([BG, T], f32, name="scb", tag="scb")
            nc.scalar.activation(scb[:], spT[:], act.Sigmoid)
            scf = scpool.tile([1, NBT], f32, name="scf", tag="scf")
            nc.gpsimd.dma_start(scf[0:1, :], scb[:])
            pbc = pssc.tile([128, NBT], f32, name="pbc", tag="pbc")
            nc.tensor.matmul(pbc[:], ones_f[:], scf[:], start=True, stop=True)
            ssc = scpool.tile([128, NBT], f32, name="ssc", tag="ssc")
            nc.scalar.activation(ssc[:], pbc[:], act.Copy)
            # XT = (x + bias) * score
            for hb in range(NHB):
                nc.vector.scalar_tensor_tensor(
                    XT[:, hb * NBT:(hb + 1) * NBT],
                    XT[:, hb * NBT:(hb + 1) * NBT],
                    bias_sb[:, hb:hb + 1], ssc[:], op0=aop.add, op1=aop.mult)

        def emit_scan(g):
            XT = XTs[g]
            v = vpool.tile([128, 128], f32, name="v", tag="v")
            nc.vector.memset(v[:], 0.0)
            x4 = XT[:].rearrange("p (hb b t) -> p hb b t", hb=NHB, b=BG)
            v3 = v[:].rearrange("p (hb b) -> p hb b", hb=NHB)
            for t in range(T):
                xt = x4[:, :, :, t]
                nc.vector.tensor_tensor(xt, xt, v3, aop.add)
                nc.vector.scalar_tensor_tensor(v3, xt, VTH, xt,
                                               op0=aop.is_lt, op1=aop.mult)
            XS = xspool.tile([128, NHB * NBT], bf16, name="XS", tag="XS")
            XSs[g] = XS
            half = NHB * NBT // 2
            for piece in range(2):
                nc.vector.tensor_scalar(
                    XS[:, piece * half:(piece + 1) * half],
                    XT[:, piece * half:(piece + 1) * half],
                    VTH, None, op0=aop.is_ge)

        def emit_output(g):
            XS = XSs[g]
            for c in range(BG // 2):
                OS = ospool.tile([128, DH], bf16, name="OS", tag="OS")
                for hb in range(NHB):
                    pt = ptr.tile([128, 128], bf16, name="pt", tag="pt")
                    nc.tensor.transpose(
                        pt[:], XS[:, hb * NBT + c * 128: hb * NBT + (c + 1) * 128],
                        ident_bf[:])
                    nc.scalar.activation(OS[:, hb * 128:(hb + 1) * 128], pt[:],
                                         act.Copy)
                b0 = g * BG + 2 * c
                nc.sync.dma_start(out[b0:b0 + 2, :, :], OS[:])

        for g in range(NG):
            emit_loads(g)
            M1 = emit_gemm(g)
            # output stage of group g-2 fills the TensorE gap while the DVE
            # stats chain for group g drains
            if g >= 2:
                emit_output(g - 2)
            emit_stats_score(g, M1)
            emit_scan(g)
        emit_output(NG - 2)
        emit_output(NG - 1)


_CACHE = {}


def _get_compiled(mode=MODE, bg=BG):
    key = (mode, bg)
    if key in _CACHE:
        return _CACHE[key]
    import concourse.tile as tile
    from concourse import bacc, mybir
    nc = bacc.Bacc("TRN2", target_bir_lowering=False, debug=False,
                   num_devices=1)
    _build(nc, tile, mybir)
    nc.compile()
    _CACHE[key] = nc
    return nc


def _prep_weights(W, bias, W1, W2):
    import ml_dtypes
    bf = ml_dtypes.bfloat16
    Whi = W.astype(bf).astype(np.float32)          # [DH, DIN]
    Wlo = (W - Whi).astype(bf).astype(np.float32)
    WEd = np.concatenate([Whi.T, Wlo.T], axis=0)   # [2048 k, 2048 h]
    wT = np.ascontiguousarray(
        WEd.reshape(NKS, 128, NHB, 128).transpose(2, 1, 0, 3)
    ).reshape(NHB, 128, NKS * 128).astype(bf)
    wavg = W.mean(axis=0, dtype=np.float64).astype(np.float32)  # [DIN]
    whi = wavg.astype(bf).astype(np.float32)
    wlo = (wavg - whi).astype(bf).astype(np.float32)
    wavgT = np.zeros((128, NKS), np.float32)
    wavgT[:, 0::2] = whi.reshape(8, 128).T
    wavgT[:, 1::2] = wlo.reshape(8, 128).T
    biasT = np.ascontiguousarray(bias.reshape(NHB, 128).T).astype(np.float32)
    mbT = np.full((128, 1), bias.mean(dtype=np.float64), np.float32)
    return dict(wT=wT, wavgT=wavgT.astype(bf), biasT=biasT, mbT=mbT,
                w1t=np.ascontiguousarray(W1.T).astype(np.float32),
                w2t=np.ascontiguousarray(W2.T).astype(np.float32))


def _prep_data_shard(shard):
    import ml_dtypes
    bf = ml_dtypes.bfloat16
    rows = shard.reshape(BS * T, DIN).astype(np.float32)
    dhi = rows.astype(bf).astype(np.float32)
    dlo = (rows - dhi).astype(bf).astype(np.float32)
    dET = np.concatenate([dhi.T, dlo.T], axis=0)   # [2048 k, 2048 bt]
    return np.ascontiguousarray(dET).reshape(NKS, 128, BS * T).astype(bf)


def _prep_all(inputs):
    data = np.asarray(inputs["data"], dtype=np.float32)
    W = np.asarray(inputs["W"], dtype=np.float32)
    bias = np.asarray(inputs["bias"], dtype=np.float32)
    W1 = np.asarray(inputs["W1"], dtype=np.float32)
    W2 = np.asarray(inputs["W2"], dtype=np.float32)
    wargs = _prep_weights(W, bias, W1, W2)
    in_maps = []
    for c in range(NCORES):
        shard = data[c * BS:(c + 1) * BS]
        in_maps.append({"dT": _prep_data_shard(shard), **wargs})
    return in_maps


def kernel(data, W, bias, W1, W2):
    from concourse.bass_utils import run_bass_kernel_spmd

    in_maps = _prep_all(dict(data=data, W=W, bias=bias, W1=W1, W2=W2))
    nc = _get_compiled()
    res = run_bass_kernel_spmd(nc, in_maps, core_ids=list(range(NCORES)))
    outs = [np.asarray(res.results[c]["out"]).astype(np.float32)
            for c in range(NCORES)]
    return np.concatenate(outs, axis=0)


if __name__ == "__main__":
    rng = np.random.default_rng(0)
    d = rng.standard_normal((B, T, DIN)).astype(np.float32)
    w = (rng.standard_normal((DH, DIN)) / 32.0).astype(np.float32)
    b = np.zeros(DH, np.float32)
    w1 = (rng.standard_normal((4, T)) / 8.0).astype(np.float32)
    w2 = (rng.standard_normal((T, 4)) / 2.0).astype(np.float32)
    o = kernel(d, w, b, w1, w2)
    print(o.shape, o.dtype, o.mean())
